# revision 1
# baseline (speedup 1.0000x reference)
"""Trainium2 Bass kernel for nn_BondMessagePassing (D-MPNN style GNN).

Contract: kernel(**inputs) takes FULL unsharded inputs (as produced by the
reference's setup_inputs) and returns the FULL output [400000, 128] float32.

Math: the reference builds edges in exact reverse pairs (edge 2k+1 is the
reverse of edge 2k, rev_edge_index = i^1), which makes dst[rev] == src.
Consequently the two scatter-adds inside every message-passing iteration
cancel exactly, so m == 0 through the loop and the output reduces to

    h   = relu([x[src], edge_attr] @ W_i)
    m   = scatter_add(h, dst)
    out = relu([x, m] @ W_o)

(biases are all zero for the documented generator; nonzero biases or a
broken reverse-pair identity fall back to an exact numpy replication).

Sharding: nodes are range-partitioned across the 8 cores (50000 nodes
each); each core receives exactly the edges whose dst lands in its range
(sorted by dst) so the scatter-add is core-local and the output rows are a
contiguous slice -- no collectives. The scatter-add runs on the
TensorEngine as matmuls against one-hot matrices over supertiles of 512
nodes: 4 window tiles (128 edges x 128-node windows) plus one overflow
tile of up to OVCAP edges with a full-width [OVCAP, 512] one-hot. All
one-hots are prebuilt fp8 on the host and DMA'd.

Perf notes (vs the first-session baseline, 184-207us):
 - every DMA'd tensor has a row count that is a multiple of 16: the HW DGE
   stripes a transfer's rows across engines in ceil(rows/16)-row groups
   starting at engine 0, so 80/64/128-row tensors use all 16 DMA engines
   (the old 81/65-row tensors left 7 of 16 engines nearly idle).
 - PSUM drains are the scalar/vector bottleneck (~1.6ns/col for any
   PSUM-source op, both engines): the on-device iota/is_equal overflow
   one-hot build (41us of vector time) is replaced by a DMA'd [80,512]
   fp8 one-hot; out-relu alternates scalar/vector; m-cast on vector,
   h-relu on scalar.
 - output DMAs issue from the scalar engine's DGE queue so they cannot
   head-of-line-block the input DMAs on the sync queue.
 - chunk schedule ramps 1,2,4,7,...,7,4,2,1 to shorten the DMA-only head
   and tail of the pipeline.
"""

import ml_dtypes
import numpy as np

# ---- problem constants (hardcoded per contract) ----
N = 400000
E = 400000
XD = 64        # node feature dim
EAD = 16       # edge feature dim
HID = 128      # hidden dim
DEPTH = 5
NCORES = 8
NL = N // NCORES          # nodes per core
SUP = 512                 # nodes per supertile (one PSUM bank of fp32)
NSUP = (NL + SUP - 1) // SUP
NPAD = NSUP * SUP         # padded nodes per core
P = 128                   # partitions / tile rows
NW = 4                    # fixed 128-node windows per supertile

F16 = np.float16
F32 = np.float32
F8 = ml_dtypes.float8_e4m3


def _check_fast_path_ok(src, dst, rev, x, edge_attr, b_i, b_h, b_o):
    """True iff the loop-cancellation identity holds, biases are zero, and
    fp16 is safe."""
    if src.shape != (E,) or dst.shape != (E,) or rev.shape != (E,):
        return False
    if np.any(b_i) or np.any(b_h) or np.any(b_o):
        return False
    if rev.min() < 0 or rev.max() >= E:
        return False
    seen = np.zeros(E, dtype=bool)
    seen[rev] = True
    if not seen.all():
        return False
    if not np.array_equal(dst[rev], src):
        return False
    if src.min() < 0 or src.max() >= N or dst.min() < 0 or dst.max() >= N:
        return False
    mx = float(np.abs(x).max(initial=0.0))
    mea = float(np.abs(edge_attr).max(initial=0.0))
    if max(mx, mea) > 100.0:
        return False
    return True


def _reference_fallback(x, edge_index, edge_attr, rev_edge_index,
                        W_i, b_i, W_h, b_h, W_o, b_o):
    def san(t):
        return np.nan_to_num(t, nan=0.0, posinf=1000.0, neginf=-1000.0)

    src, dst = edge_index[0], edge_index[1]
    h0 = np.maximum(
        np.concatenate([x[src], edge_attr], axis=1) @ W_i + b_i, 0.0
    ).astype(F32)
    h = h0
    for _ in range(1, DEPTH):
        m = np.zeros_like(h)
        np.add.at(m, dst, h)
        np.add.at(m, src, -h[rev_edge_index])
        m = san(m) @ W_h + b_h
        h = np.maximum(h0 + m, 0.0).astype(F32)
    m_final = np.zeros_like(h)
    np.add.at(m_final, dst, h)
    h_cat = np.concatenate([x, san(m_final)], axis=1)
    out = np.maximum(h_cat @ W_o + b_o, 0.0).astype(F32)
    return san(out)


_PROGRAM_CACHE = {}


def _build_program(OVCAP):
    """Build the SPMD Bass program.

    OVCAP = overflow-tile capacity per supertile (multiple of 16, <= 128).
    Per-supertile zt layout: 4 window tiles of 128 slots + OVCAP ov slots.
    """
    import concourse.bacc as bacc
    import concourse.mybir as mybir
    import concourse.tile as tile

    nc = bacc.Bacc("TRN2", target_bir_lowering=False, debug=False,
                   num_devices=NCORES)
    dt = mybir.dt
    SLOT = NW * P + OVCAP              # zt cols per supertile

    zt = nc.dram_tensor("zt", [80, NSUP * SLOT], dt.float16,
                        kind="ExternalInput")
    xct = nc.dram_tensor("xct", [XD, NPAD], dt.float8e4,
                         kind="ExternalInput")
    s4d = nc.dram_tensor("s4d", [P, NSUP * NW * P], dt.float8e4,
                         kind="ExternalInput")
    sov = nc.dram_tensor("sov", [OVCAP, NSUP * SUP], dt.float8e4,
                         kind="ExternalInput")
    w_ih = nc.dram_tensor("w_ih", [80, HID], dt.float16, kind="ExternalInput")
    w_o1 = nc.dram_tensor("w_o1", [XD, HID], dt.float16, kind="ExternalInput")
    w_o2 = nc.dram_tensor("w_o2", [HID, HID], dt.float16, kind="ExternalInput")
    # output produced TRANSPOSED ([hidden, node]); host transposes back
    outT = nc.dram_tensor("outT", [HID, NPAD], dt.float16,
                          kind="ExternalOutput")

    RELU = mybir.ActivationFunctionType.Relu

    with tile.TileContext(nc) as tc:
        with (
            tc.tile_pool(name="consts", bufs=1) as consts,
            tc.tile_pool(name="ztp", bufs=3) as ztp,
            tc.tile_pool(name="xctp", bufs=3) as xctp,
            tc.tile_pool(name="sp", bufs=3) as sp,
            tc.tile_pool(name="sovp", bufs=3) as sovp,
            tc.tile_pool(name="hp", bufs=4) as hp,
            tc.tile_pool(name="hpo", bufs=2) as hpo,
            tc.tile_pool(name="mp", bufs=4) as mp,
            tc.tile_pool(name="op", bufs=3) as op,
            tc.tile_pool(name="hps", bufs=2, space="PSUM") as hps,
            tc.tile_pool(name="hops", bufs=1, space="PSUM") as hops,
            tc.tile_pool(name="mps", bufs=3, space="PSUM") as mps,
            tc.tile_pool(name="ops", bufs=2, space="PSUM") as ops,
        ):
            w_ih_t = consts.tile([80, HID], dt.float16)
            nc.sync.dma_start(out=w_ih_t, in_=w_ih[:, :])
            w_o1_t = consts.tile([XD, HID], dt.float16)
            nc.sync.dma_start(out=w_o1_t, in_=w_o1[:, :])
            w_o2_t = consts.tile([HID, HID], dt.float16)
            nc.sync.dma_start(out=w_o2_t, in_=w_o2[:, :])

            # chunk schedule: ramp up fast, steady 7s, ramp down
            sched = []
            t0 = 0
            for g in (1, 2, 4):
                sched.append((t0, g)); t0 += g
            while NSUP - t0 > 7:
                sched.append((t0, 7)); t0 += 7
            for g in (4, 2, 1):
                if NSUP - t0 >= g:
                    sched.append((t0, g)); t0 += g
            while t0 < NSUP:
                sched.append((t0, 1)); t0 += 1
            assert sum(g for _, g in sched) == NSUP

            for TT, Gc in sched:
                zt_c = ztp.tile([80, Gc * SLOT], dt.float16, tag="ztc")
                nc.sync.dma_start(
                    out=zt_c, in_=zt[:, TT * SLOT:(TT + Gc) * SLOT])
                s4_c = sp.tile([P, Gc * NW * P], dt.float8e4, tag="s4c")
                nc.sync.dma_start(
                    out=s4_c, in_=s4d[:, TT * NW * P:(TT + Gc) * NW * P])
                sov_c = sovp.tile([OVCAP, Gc * SUP], dt.float8e4, tag="sovc")
                nc.sync.dma_start(
                    out=sov_c, in_=sov[:, TT * SUP:(TT + Gc) * SUP])
                xct_c = xctp.tile([XD, Gc * SUP], dt.float8e4, tag="xctc")
                nc.sync.dma_start(
                    out=xct_c, in_=xct[:, TT * SUP:(TT + Gc) * SUP])
                o_buf = op.tile([P, Gc * SUP], dt.float16, tag="obuf")

                # --- overflow h0 for the chunk, grouped 4 supertiles/bank ---
                h_sbO = hpo.tile([OVCAP, Gc * HID], dt.float16, tag="hsbo")
                for base in range(0, Gc, 4):
                    cnt = min(4, Gc - base)
                    h_pso = hops.tile([OVCAP, cnt * HID], mybir.dt.float32,
                                      tag="hpso")
                    for g2 in range(cnt):
                        nc.tensor.matmul(
                            h_pso[:, g2 * HID:(g2 + 1) * HID],
                            zt_c[:, (base + g2) * SLOT + NW * P:
                                 (base + g2) * SLOT + NW * P + OVCAP],
                            w_ih_t, start=True, stop=True)
                    # overflow h relu: vector engine
                    nc.vector.tensor_scalar_max(
                        h_sbO[:, base * HID:(base + cnt) * HID], h_pso, 0.0)

                for g in range(Gc):
                    T = TT + g
                    xctg = xct_c[:, g * SUP:(g + 1) * SUP]

                    # --- h0: 4 window tiles in one bank ---
                    h_ps4 = hps.tile([P, NW * HID], mybir.dt.float32)
                    for jj in range(NW):
                        nc.tensor.matmul(
                            h_ps4[:, jj * HID:(jj + 1) * HID],
                            zt_c[:, g * SLOT + jj * P:g * SLOT + (jj + 1) * P],
                            w_ih_t, start=True, stop=True)
                    h_sb = hp.tile([P, NW * HID], dt.float16)
                    nc.scalar.activation(h_sb, h_ps4, RELU)

                    # --- scatter-add on the PE: m_T[h, n] += h^T @ S ---
                    m_psum = mps.tile([P, SUP], mybir.dt.float32)
                    nc.tensor.matmul(m_psum,
                                     h_sbO[:, g * HID:(g + 1) * HID],
                                     sov_c[:, g * SUP:(g + 1) * SUP],
                                     start=True, stop=False,
                                     skip_group_check=True)
                    s4g = s4_c[:, g * NW * P:(g + 1) * NW * P]
                    for jj in range(NW):
                        nc.tensor.matmul(m_psum[:, jj * P:(jj + 1) * P],
                                         h_sb[:, jj * HID:(jj + 1) * HID],
                                         s4g[:, jj * P:(jj + 1) * P],
                                         start=False, stop=(jj == NW - 1),
                                         skip_group_check=True)

                    # m cast: vector engine
                    m_t = mp.tile([P, SUP], dt.float16)
                    nc.vector.tensor_copy(m_t, m_psum)

                    # --- out^T[o, v] = relu(W_o1^T @ xct + W_o2^T @ m_T) ---
                    o_psum = ops.tile([P, SUP], mybir.dt.float32)
                    nc.tensor.matmul(o_psum, w_o1_t, xctg,
                                     start=True, stop=False)
                    nc.tensor.matmul(o_psum, w_o2_t, m_t,
                                     start=False, stop=True)
                    # out relu: alternate scalar / vector to balance drains
                    ob = o_buf[:, g * SUP:(g + 1) * SUP]
                    if T % 2 == 0:
                        nc.scalar.activation(ob, o_psum, RELU)
                    else:
                        nc.vector.tensor_scalar_max(ob, o_psum, 0.0)

                # output DMA from the scalar queue (not sync): input DMAs
                # for later chunks never queue behind this compute-gated one
                nc.scalar.dma_start(out=outT[:, TT * SUP:(TT + Gc) * SUP],
                                    in_=o_buf)

    nc.compile()
    return nc


def kernel(**inputs):
    x = np.ascontiguousarray(np.asarray(inputs["x"]), dtype=F32)
    edge_index = np.asarray(inputs["edge_index"]).astype(np.int64)
    edge_attr = np.ascontiguousarray(np.asarray(inputs["edge_attr"]), dtype=F32)
    rev = np.asarray(inputs["rev_edge_index"]).astype(np.int64)
    W_i = np.asarray(inputs["W_i"], dtype=F32)
    b_i = np.asarray(inputs["b_i"], dtype=F32)
    W_h = np.asarray(inputs["W_h"], dtype=F32)
    b_h = np.asarray(inputs["b_h"], dtype=F32)
    W_o = np.asarray(inputs["W_o"], dtype=F32)
    b_o = np.asarray(inputs["b_o"], dtype=F32)

    src, dst = edge_index[0], edge_index[1]

    if not _check_fast_path_ok(src, dst, rev, x, edge_attr, b_i, b_h, b_o):
        return _reference_fallback(x, edge_index, edge_attr, rev,
                                   W_i, b_i, W_h, b_h, W_o, b_o)

    from concourse.bass_utils import run_bass_kernel_spmd

    # ---- host-side graph partition / sort (indices only) ----
    order = np.argsort(dst, kind="stable")
    dst_s = dst[order]
    core_starts = np.searchsorted(dst_s, np.arange(0, N + NL, NL))

    per_core = []
    max_ov_all = 0
    for c in range(NCORES):
        e0, e1 = core_starts[c], core_starts[c + 1]
        ne = e1 - e0
        ld = dst_s[e0:e1] - c * NL           # local dst, sorted
        gidx = ld // P                       # 128-node window id
        gstarts = np.searchsorted(ld, np.arange(0, NPAD + P, P))
        r = np.arange(ne) - gstarts[gidx]    # rank within window
        wmask = r < P
        T = ld // SUP                        # supertile id
        tstarts = np.searchsorted(ld, np.arange(0, NPAD + SUP, SUP))
        ocum = np.cumsum(~wmask)
        prefix = np.concatenate(([0], ocum))
        o_rank = (ocum - 1) - prefix[tstarts[T]]
        n_ov = int(np.sum(~wmask))
        max_ov = int(o_rank[~wmask].max()) + 1 if n_ov else 0
        max_ov_all = max(max_ov_all, max_ov)
        per_core.append((e0, e1, ld, gidx, r, wmask, T, o_rank))

    # one overflow tile (<=128 edges per supertile); beyond that, exact
    # numpy fallback (cannot happen for the documented uniform generator)
    if max_ov_all > P:
        return _reference_fallback(x, edge_index, edge_attr, rev,
                                   W_i, b_i, W_h, b_h, W_o, b_o)
    OVCAP = max(80, -(-max_ov_all // 16) * 16)
    SLOT = NW * P + OVCAP

    # ---- shared constant tensors ----
    w_ih_np = np.ascontiguousarray(W_i).astype(F16)          # [80,128]
    w_o1_np = np.ascontiguousarray(W_o[:XD]).astype(F16)     # [64,128]
    w_o2_np = np.ascontiguousarray(W_o[XD:]).astype(F16)     # [128,128]

    x16t = np.ascontiguousarray(x.T.astype(F16))             # [64, N]
    ea16t = np.ascontiguousarray(edge_attr.T.astype(F16))    # [16, E]

    in_maps = []
    for c in range(NCORES):
        e0, e1, ld, gidx, r, wmask, T, o_rank = per_core[c]
        eids = order[e0:e1]

        # slot per edge: window edges -> window tile gidx%4 of supertile T
        # at rank r; overflow edges -> ov slots 512..512+OVCAP.
        slots = np.where(
            wmask,
            T * SLOT + (gidx % NW) * P + r,
            T * SLOT + NW * P + o_rank,
        )

        zt_np = np.zeros((80, NSUP * SLOT), dtype=F16)
        zt_np[0:XD, slots] = x16t[:, src[eids]]
        zt_np[XD:XD + EAD, slots] = ea16t[:, eids]

        # window one-hots, fp8, [rank, T*(4*128) + win*128 + local]
        s4_np = np.zeros((P, NSUP * NW * P), dtype=F8)
        ws = wmask
        s4_np[r[ws], T[ws] * (NW * P) + (gidx[ws] % NW) * P + (ld[ws] % P)] = 1.0

        # overflow one-hots, fp8, [o_rank, T*512 + local512]
        sov_np = np.zeros((OVCAP, NSUP * SUP), dtype=F8)
        ovs = ~wmask
        sov_np[o_rank[ovs], T[ovs] * SUP + (ld[ovs] % SUP)] = 1.0

        xct_np = np.zeros((XD, NPAD), dtype=F8)
        xct_np[:, :NL] = x[c * NL:(c + 1) * NL].T.astype(F8)

        in_maps.append({
            "zt": zt_np, "xct": xct_np, "s4d": s4_np, "sov": sov_np,
            "w_ih": w_ih_np, "w_o1": w_o1_np, "w_o2": w_o2_np,
        })

    if OVCAP not in _PROGRAM_CACHE:
        _PROGRAM_CACHE[OVCAP] = _build_program(OVCAP)
    nc = _PROGRAM_CACHE[OVCAP]

    import os
    trace = bool(os.environ.get("BMP_TRACE"))
    res = run_bass_kernel_spmd(nc, in_maps, core_ids=list(range(NCORES)),
                               trace=trace)
    if trace:
        global LAST_EXEC_TIME_NS, LAST_TRACE
        LAST_EXEC_TIME_NS = res.exec_time_ns
        LAST_TRACE = res.instructions_and_trace
    out = np.empty((N, HID), dtype=F32)
    for c in range(NCORES):
        out[c * NL:(c + 1) * NL] = res.results[c]["outT"][:, :NL].T.astype(F32)
    return out

